# revision 49
# baseline (speedup 1.0000x reference)
"""Multi-head self-attention (B=2, S=2048, D=1024, H=16) on 8 TRN2 NeuronCores.

Sharding: head-parallel - 2 heads per core. Each core computes Q/K/V
projections for its 2 heads over all B*S tokens, full (non-causal)
softmax attention for its 4 (batch, head) units, and a partial output
projection y_c = sum_h out_h @ wo[h]. Host sums the 8 partial outputs.
The host pre-transposes x to xT and casts everything to bf16.

Device dataflow (all matmul operands bf16, PSUM accumulation fp32):
  q2t/k2t [128=2*64, T]  = w[:,2heads]^T @ xT       (PSUM accum over D)
  v --PE transpose--> vnat[k, h, 0:64] (+ ones column at col 64)
  scoresT[k, q]: per (kt, qc) window TWO concurrent row-tiled matmuls
    (head h uses array rows h*64..h*64+63 via partition-sliced APs ->
    tile_position (h*64, 0)); both heads' [128,512] tiles land in one
    2-bank PSUM tile.
  ex = exp(scale * sc) on ACT: ONE [128, 1024] activation per window.
  AV ex-stationary: po[128 q, 65] += ex_tile[128k,128q].T @ vnat[128k,65]
    accumulated over kt. Column 64 (ones) gives the softmax denominator
    per-query-ON-PARTITIONS -> reciprocal is a [128,1] DVE op (cheap).
  att[q, h*64+d] = po[:,0:64] * rci  (per-partition scalar mul)
  out2t[:, q] = PE transpose of att [128,128] (both heads at once)
  y[t, n] = out2t[:, ttile]^T @ wo   (contract 128 = 2 heads * 64)

Emission is window-scheduled: per (batch, qchunk) block there are KT
score+exp windows paced by the ACT engine (~1 us each); projection
matmuls, the previous block's AV groups, drains, transposes and output
projection tiles are spread across windows as fillers so the PE always
has work while ACT grinds through the exps.
"""

import numpy as np
from contextlib import ExitStack

import concourse.bass as bass
import concourse.tile as tile
from concourse import bacc, mybir
from concourse.bass_utils import run_bass_kernel_spmd
from concourse.masks import make_identity

F32 = mybir.dt.float32
BF = mybir.dt.bfloat16
AF = mybir.ActivationFunctionType

N_CORES = 8
D_MODEL = 1024
NUM_HEADS = 16
DEPTH = 64
HEADS_PER_CORE = NUM_HEADS // N_CORES  # 2
B_FULL = 2
S_FULL = 2048


def build_program(T=4096, D=1024, S=2048, dh=64, hc=2, with_qkv_bias=False,
                  with_o_bias=False, dbg=False, rowtile=True, ex_fp8=False):
    """Build the SPMD Bass program for one core (hc heads).

    T: total tokens (B*S); D: model dim; S: seq len per batch; dh: head
    depth; hc: heads per core. Requires hc*dh == 128, D % 128 == 0,
    S % 512 == 0, T % S == 0.
    """
    d2 = hc * dh
    assert d2 == 128 and D % 128 == 0 and S % 512 == 0 and T % S == 0
    nb = T // S            # batches
    ndc = D // 128         # D chunks (contraction tiles)
    cpb = S // 512         # 512-token chunks per batch
    KT = S // 128          # k tiles (and windows) per block
    QC = S // 512          # 512-wide q chunks (blocks) per batch
    QT = 512 // 128        # 128-wide q tiles per block
    NG = hc * QT           # AV accumulation groups per block
    NJ = min(512, D)
    ndj = D // NJ
    nch = nb * cpb         # total xt chunks
    nblocks = nb * QC
    scale = 1.0 / float(np.sqrt(dh))
    # ex dtype: fp8e4 (with FWL 4x weight load for the ex-stationary AV
    # matmuls). exp is shifted by EXB so the fp8e4 max normal (240) is never
    # hit: softmax is shift-invariant (numerator and the ones-column
    # denominator are scaled identically).
    EXDT = mybir.dt.float8e4 if ex_fp8 else BF
    EXB = -1.5 if ex_fp8 else 0.0

    nc = bacc.Bacc("TRN2", target_bir_lowering=False, debug=False,
                   num_devices=N_CORES)

    xt_d = nc.dram_tensor("xt", [D, T], BF, kind="ExternalInput").ap()
    wq_d = nc.dram_tensor("wq", [D, d2], BF, kind="ExternalInput").ap()
    wk_d = nc.dram_tensor("wk", [D, d2], BF, kind="ExternalInput").ap()
    wv_d = nc.dram_tensor("wv", [D, d2], BF, kind="ExternalInput").ap()
    wo_d = nc.dram_tensor("wo", [d2, D], BF, kind="ExternalInput").ap()
    if with_qkv_bias:
        bq_d = nc.dram_tensor("bq", [d2, 1], F32, kind="ExternalInput").ap()
        bk_d = nc.dram_tensor("bk", [d2, 1], F32, kind="ExternalInput").ap()
        bv_d = nc.dram_tensor("bv", [d2, 1], F32, kind="ExternalInput").ap()
    if with_o_bias:
        bo_d = nc.dram_tensor("bo", [1, D], F32, kind="ExternalInput").ap()
    y_d = nc.dram_tensor("y", [T, D], BF, kind="ExternalOutput").ap()
    if dbg:
        dq2t_d = nc.dram_tensor("dq2t", [128, T], BF,
                                kind="ExternalOutput").ap()
        dk2t_d = nc.dram_tensor("dk2t", [128, T], BF,
                                kind="ExternalOutput").ap()
        dvnat_d = nc.dram_tensor("dvnat", [128, nb * KT * hc * (dh + 1)], BF,
                                 kind="ExternalOutput").ap()
        dex_d = nc.dram_tensor("dex", [128, hc * 512], EXDT,
                               kind="ExternalOutput").ap()
        datt_d = nc.dram_tensor("datt", [128, 128], BF,
                                kind="ExternalOutput").ap()
        dout2t_d = nc.dram_tensor("dout2t", [128, T], BF,
                                  kind="ExternalOutput").ap()

    xt_view = xt_d.rearrange("(dc p) t -> p dc t", p=128)

    with tile.TileContext(nc) as tc, ExitStack() as ctx:
        singles = ctx.enter_context(tc.tile_pool(name="singles", bufs=1))
        xtpool = ctx.enter_context(tc.tile_pool(name="xtpool", bufs=1))
        v2pool = ctx.enter_context(tc.tile_pool(name="v2pool", bufs=2))
        expool = ctx.enter_context(tc.tile_pool(name="expool", bufs=2 * KT))
        attpool = ctx.enter_context(tc.tile_pool(name="attpool", bufs=4))
        rcpool = ctx.enter_context(tc.tile_pool(name="rcpool", bufs=4))
        ysb = ctx.enter_context(tc.tile_pool(name="ysb", bufs=6))
        # PSUM budget (8 banks): sc 2x[128,2,512]=4, po 1x[128,NG,128]=2,
        # ps 2x[128,512]=2
        scpool = ctx.enter_context(tc.tile_pool(name="sc", bufs=2,
                                                space="PSUM"))
        popool = ctx.enter_context(tc.tile_pool(name="po", bufs=1,
                                                space="PSUM"))
        pspool = ctx.enter_context(tc.tile_pool(name="ps", bufs=2,
                                                space="PSUM"))

        identb = singles.tile([128, 128], BF)
        make_identity(nc, identb[:])
        exbias = None
        if EXB != 0.0:
            exbias = singles.tile([128, 1], F32)
            nc.vector.memset(exbias[:], EXB)

        # Weights land as bf16. One dma_start = one DMA queue (~24 GB/s), so
        # ramp-critical tensors (wq/wk, first chunks) are split per-dc to
        # spread across queues, and issues alternate between the sync and
        # gpsimd sequencers (each DMA issue costs ~600ns of sequencer time).
        _eng = [nc.sync, nc.gpsimd, nc.scalar]
        _ei = [0]
        _nel = [3]

        def dma(out, in_):
            # scalar (ACT) can issue DMAs too - use it only in the preamble,
            # before the exp stream starts
            _eng[_ei[0] % _nel[0]].dma_start(out=out, in_=in_)
            _ei[0] += 1

        w_sb = []
        w_views = []
        for name, wd in (("wqs", wq_d), ("wks", wk_d), ("wvs", wv_d)):
            t = singles.tile([128, ndc, d2], BF, tag=name, name=name)
            w_sb.append(t)
            w_views.append(wd.rearrange("(dc p) m -> p dc m", p=128))
        wo_sb = singles.tile([d2, D], BF)

        b_sb = [None, None, None]
        if with_qkv_bias:
            for i, bd in enumerate((bq_d, bk_d, bv_d)):
                t = singles.tile([d2, 1], F32, tag=f"b{i}", name=f"b{i}")
                nc.sync.dma_start(out=t[:], in_=bd)
                b_sb[i] = t
        bo_sb = None
        if with_o_bias:
            bo_sb = singles.tile([128, D], F32)
            nc.gpsimd.dma_start(out=bo_sb[:], in_=bo_d.partition_broadcast(128))

        if rowtile:
            q2t = singles.tile([128, T], BF, tag="q2t")
        else:
            # padded fallback: per-head Q with the other head's rows zeroed;
            # scores contract K=128 with the full two-head K tile stationary.
            q2tz = [singles.tile([128, T], BF, tag=f"q2tz{h}",
                                 name=f"q2tz{h}") for h in range(hc)]
            for h in range(hc):
                zrows = (slice(dh, 128) if h == 0 else slice(0, h * dh))
                nc.vector.memset(q2tz[h][zrows, :], 0.0)
        k2t = singles.tile([128, T], BF, tag="k2t")
        out2t = singles.tile([128, T], BF, tag="out2t")
        # vnat[:, b, kt, h, 0:64] = V rows (k on partitions); col 64 = ones
        vnat = singles.tile([128, nb, KT, hc, dh + 1], BF, tag="vnat")
        onesc = singles.tile([128, nb, KT, hc, 1], BF)
        nc.vector.memset(onesc[:], 1.0)
        nc.vector.tensor_copy(vnat[:, :, :, :, dh:dh + 1], onesc[:])

        xts = {}     # chunk idx -> xt tile
        exs = {}     # (b, qc, kt) -> ex tile
        po_t = {}    # block idx -> po psum tile
        att_t = {}   # (b, qc, qt) -> att sbuf tile

        # PE warm-up: ~3.4us of dependency-free matmuls issued while the
        # first DMAs are in flight trips the HAM clock-gate to 2.4 GHz, so
        # the DMA-gated projection work that follows runs at full speed.
        wsc = singles.tile([128, 512], BF)
        nc.vector.memset(wsc[:], 0.0)
        wps = pspool.tile([128, 512], F32, tag="ps", name="warm")
        for _ in range(14):
            nc.tensor.matmul(wps[:], identb[:], wsc[:], start=True,
                             stop=True)

        # ---------- emission helpers ----------
        def chunk_dma(n, step=1):
            """Load xt chunk n; step=1 -> per-dc transfers (8 queues in
            parallel, for ramp-critical chunks), step=2 -> dc-pairs."""
            xt_n = xtpool.tile([128, ndc, 512], BF, tag=f"xt{n}",
                               name=f"xt{n}")
            for dc in range(0, ndc, step):
                dma(xt_n[:, dc:dc + step, :],
                    xt_view[:, dc:dc + step, n * 512:(n + 1) * 512])
            xts[n] = xt_n

        def drain_proj(p, n, ps):
            """Drain projection p of chunk n from PSUM to its home."""
            ncol = slice(n * 512, (n + 1) * 512)
            if p == 0:
                if rowtile:
                    if with_qkv_bias:
                        nc.vector.tensor_scalar_add(q2t[:, ncol], ps[:],
                                                    b_sb[0][:])
                    else:
                        nc.vector.tensor_copy(q2t[:, ncol], ps[:])
                else:
                    for h in range(hc):
                        hp = slice(h * dh, (h + 1) * dh)
                        if with_qkv_bias:
                            nc.vector.tensor_scalar_add(
                                q2tz[h][hp, ncol], ps[hp, :],
                                b_sb[0][hp, :])
                        else:
                            nc.vector.tensor_copy(q2tz[h][hp, ncol],
                                                  ps[hp, :])
            elif p == 1:
                if with_qkv_bias:
                    nc.vector.tensor_scalar_add(k2t[:, ncol], ps[:],
                                                b_sb[1][:])
                else:
                    nc.vector.tensor_copy(k2t[:, ncol], ps[:])
            else:
                v2 = v2pool.tile([128, 512], BF, tag="v2", name=f"v2_{n}")
                if with_qkv_bias:
                    nc.vector.tensor_scalar_add(v2[:], ps[:], b_sb[2][:])
                else:
                    nc.vector.tensor_copy(v2[:], ps[:])
                bb, kt0 = n // cpb, (n % cpb) * 4
                pv = pspool.tile([128, 4, 128], BF, tag="ps", name=f"pv{n}")
                for sub in range(4):
                    nc.tensor.transpose(pv[:, sub, :],
                                        v2[:, sub * 128:(sub + 1) * 128],
                                        identb[:])
                for h in range(hc):
                    nc.vector.tensor_copy(
                        vnat[:, bb, kt0:kt0 + 4, h, 0:dh],
                        pv[:, :, h * dh:(h + 1) * dh])

        def proj_closures(n, projs):
            """Per-(proj, dc) matmul closures for chunk n; drains attached
            to the last dc of each projection."""
            out = []
            for p in projs:
                state = {}

                def mk(p=p, n=n, state=state):
                    def run_dc(dc):
                        if dc == 0:
                            state["ps"] = pspool.tile(
                                [128, 512], F32, tag="ps", name=f"pj{n}_{p}")
                        nc.tensor.matmul(state["ps"][:], w_sb[p][:, dc, :],
                                         xts[n][:, dc, :],
                                         start=(dc == 0),
                                         stop=(dc == ndc - 1))
                        if dc == ndc - 1:
                            drain_proj(p, n, state["ps"])
                    return run_dc
                run_dc = mk()
                for dc in range(ndc):
                    out.append(lambda dc=dc, f=run_dc: f(dc))
            return out

        def window(b, qc, kt):
            """Score pair (row-tiled, both heads) + joint exp."""
            sc = scpool.tile([128, hc, 512], F32, tag="sc", bufs=2,
                             name=f"sc{b}_{qc}_{kt}")
            kcol = slice(b * S + kt * 128, b * S + (kt + 1) * 128)
            qcol = slice(b * S + qc * 512, b * S + (qc + 1) * 512)
            for h in range(hc):
                if rowtile:
                    hp = slice(h * dh, (h + 1) * dh)
                    nc.tensor.matmul(sc[:, h, :], k2t[hp, kcol],
                                     q2t[hp, qcol], start=True, stop=True)
                else:
                    nc.tensor.matmul(sc[:, h, :], k2t[:, kcol],
                                     q2tz[h][:, qcol], start=True, stop=True)
            ex = expool.tile([128, hc, 512], EXDT, tag="ex", bufs=2 * KT,
                             name=f"ex{b}_{qc}_{kt}")
            nc.scalar.activation(ex[:, :, :], sc[:, :, :], AF.Exp,
                                 scale=scale,
                                 bias=(exbias[:] if exbias is not None
                                       else 0.0))
            exs[(b, qc, kt)] = ex
            if dbg and (b, qc, kt) == (0, 0, 0):
                nc.sync.dma_start(
                    out=dex_d.rearrange("p (h q) -> p h q", h=hc),
                    in_=ex[:, :, :])

        def av_group(b, qc, g, drain=True):
            """AV accumulation group g = qt*hc + h of block (b, qc), plus
            its drain. Consecutive groups alternate between two 1-bank po
            tiles so a group's matmuls never pick up a false (tile-granular)
            dependency on the previous group's DVE drain."""
            i = b * QC + qc
            if i not in po_t:
                po_t[i] = [popool.tile([128, NG // 2, 128], F32,
                                       tag=f"po{p}", name=f"po{p}_{i}")
                           for p in range(2)]
            po = po_t[i][g % 2]
            gs = g // 2
            qt, h = g // hc, g % hc
            for kt in range(KT):
                nc.tensor.matmul(
                    po[:, gs, 0:dh + 1],
                    exs[(b, qc, kt)][:, h, qt * 128:(qt + 1) * 128],
                    vnat[:, b, kt, h, :],
                    start=(kt == 0), stop=(kt == KT - 1))
            if drain:
                rci = rcpool.tile([128, 1], F32, tag="rci", bufs=4,
                                  name=f"rci{i}_{g}")
                nc.vector.reciprocal(rci[:], po[:, gs, dh:dh + 1])
                att = get_att(b, qc, qt)
                nc.vector.tensor_scalar_mul(att[:, h * dh:(h + 1) * dh],
                                            po[:, gs, 0:dh], rci[:])

        def get_att(b, qc, qt):
            i = b * QC + qc
            if (b, qc, qt) not in att_t:
                att_t[(b, qc, qt)] = attpool.tile(
                    [128, 128], BF, tag="att", bufs=4, name=f"att{i}_{qt}")
            return att_t[(b, qc, qt)]

        def tail_drains(b, qc):
            """Batched drain of all NG groups: one strided reciprocal per po
            tile, then per-group normalize-muls on the (now idle) ACT engine
            interleaved with DVE - only used in the tail."""
            i = b * QC + qc
            rci = rcpool.tile([128, 2, NG // 2, 1], F32, tag="rcib", bufs=1,
                              name=f"rcib{i}")
            for p in range(2):
                nc.vector.reciprocal(rci[:, p, :, :],
                                     po_t[i][p][:, :, dh:dh + 1])
            for g in range(NG):
                po = po_t[i][g % 2]
                gs = g // 2
                qt, h = g // hc, g % hc
                att = get_att(b, qc, qt)
                if g % 2 == 0:
                    nc.scalar.activation(att[:, h * dh:(h + 1) * dh],
                                         po[:, gs, 0:dh], AF.Copy,
                                         scale=rci[:, g % 2, gs, :])
                else:
                    nc.vector.tensor_scalar_mul(att[:, h * dh:(h + 1) * dh],
                                                po[:, gs, 0:dh],
                                                rci[:, g % 2, gs, :])

        def av_tp(b, qc, qt):
            """Transpose the finished two-head att tile into out2t."""
            i = b * QC + qc
            att = att_t[(b, qc, qt)]
            if dbg and (b, qc, qt) == (0, 0, 0):
                nc.sync.dma_start(out=datt_d, in_=att[:])
            q0 = qc * 512 + qt * 128
            pt = pspool.tile([128, 128], BF, tag="ps", name=f"pt{i}_{qt}")
            nc.tensor.transpose(pt[:], att[:], identb[:])
            nc.vector.tensor_copy(out2t[:, b * S + q0:b * S + q0 + 128],
                                  pt[:])

        _tpy = [0]

        def oproj_j(b, tt, j, use_act=False):
            """One n-chunk of the output projection for token tile tt."""
            tcol = slice(b * S + tt * 128, b * S + (tt + 1) * 128)
            trow = slice(b * S + tt * 128, b * S + (tt + 1) * 128)
            if True:
                if use_act:
                    # tail: the AV po banks are free - rotate py over
                    # ps/po0/po1 so the oproj matmuls don't drain-serialize
                    tags = ("ps", "po0", "po1")
                    tag = tags[_tpy[0] % 3]
                    _tpy[0] += 1
                    pool = pspool if tag == "ps" else popool
                else:
                    tag, pool = "ps", pspool
                py = pool.tile([128, NJ], F32, tag=tag,
                               name=f"py{b}_{tt}_{j}")
                nc.tensor.matmul(py[:], out2t[:, tcol],
                                 wo_sb[:, j * NJ:(j + 1) * NJ],
                                 start=True, stop=True)
                yt = ysb.tile([128, NJ], BF, tag="yt", name=f"yt{b}_{tt}_{j}")
                if with_o_bias:
                    nc.vector.tensor_add(yt[:], py[:],
                                         bo_sb[:, j * NJ:(j + 1) * NJ])
                elif use_act and j % 2 == 0:
                    # tail only: ACT is idle once the exps are done; split
                    # the PSUM->SBUF drains between ACT and DVE so the py
                    # psum ring never serializes on a single engine
                    nc.scalar.copy(yt[:], py[:])
                else:
                    nc.vector.tensor_copy(yt[:], py[:])
                # split transfers over DMA queues to cut the tile's latency
                # (4-way in the tail, where the last transfer gates teardown)
                np_ = 4 if (use_act and NJ >= 512) else 2
                hj = NJ // np_
                for p in range(np_):
                    eng = nc.gpsimd if p % 2 == 0 else nc.sync
                    eng.dma_start(
                        out=y_d[trow, j * NJ + p * hj:j * NJ + (p + 1) * hj],
                        in_=yt[:, p * hj:(p + 1) * hj])

        # ---------- static schedule ----------
        # fillers[i] = list of (deadline_window_or_None, closure) for block i;
        # fillers[nblocks] is the tail.
        fillers = [[] for _ in range(nblocks + 1)]

        def add(i, cl, dl=None):
            fillers[min(i, nblocks)].append((dl, cl))

        # All input DMAs issue up front, in need-order: chunk 0/1 and wv
        # per-dc (ramp critical), later chunks coarser. Chunk 0 K+Q compute
        # is in the preamble (needed by window (0,0,0)); chunk-0 V rides
        # inside block 0 (first needed by the AV groups during block 1).
        preamble = []

        def ramp_dma():
            # wq/wk (per dc-pair) interleaved with chunk-0 (per half-dc,
            # 64KB pieces ~2.9us each on a queue) so the first projection
            # matmuls unblock as early as possible
            xt0 = xtpool.tile([128, ndc, 512], BF, tag="xt0", name="xt0")
            xts[0] = xt0
            for dc in range(0, ndc, 2):
                for w in (0, 1):
                    dma(w_sb[w][:, dc:dc + 2, :], w_views[w][:, dc:dc + 2, :])
                for d in (dc, dc + 1):
                    dma(xt0[:, d, 0:256], xt_view[:, d, 0:256])
                    dma(xt0[:, d, 256:512], xt_view[:, d, 256:512])
        preamble.append(ramp_dma)

        def chunk1_dma():
            # chunk 1 in half-dc pieces too: its K-projection is a block-0
            # early-window deadline filler
            xt1 = xtpool.tile([128, ndc, 512], BF, tag="xt1", name="xt1")
            xts[1] = xt1
            for d in range(ndc):
                dma(xt1[:, d, 0:256], xt_view[:, d, 512:768])
                dma(xt1[:, d, 256:512], xt_view[:, d, 768:1024])
        if nch > 1:
            preamble.append(chunk1_dma)

        def wv_wo_dma():
            _nel[0] = 2    # stop using the scalar queue; exps start soon
            for dc in range(0, ndc, 2):
                dma(w_sb[2][:, dc:dc + 2, :], w_views[2][:, dc:dc + 2, :])
            dma(wo_sb[:], wo_d)
        preamble.append(wv_wo_dma)
        for n in range(2, nch):
            preamble.append(lambda n=n: chunk_dma(n, step=2))
        preamble += proj_closures(0, (1, 0))
        # chunk-0 V-projection rides block 0 (its vnat writes must be
        # emitted before AV(0,0) group 0, which is a block-1 window-0
        # filler - later placement would make that matmul read stale vnat)
        for cl in proj_closures(0, (2,)):
            add(0, cl)

        # batch-0 chunks 1..cpb-1: K+V into block 0 with per-window
        # deadlines (chunk c's keys are first needed at window c*KT//cpb);
        # Q-projection of chunk (b, c) goes to the block before (b, c).
        kpc = max(1, KT // cpb)
        for c in range(1, cpb):
            kv = proj_closures(c, (1, 2))
            ws = list(range((c - 1) * kpc, c * kpc))
            for j, cl in enumerate(kv):
                add(0, cl, dl=ws[j * len(ws) // len(kv)])
        # batch>=1 chunks: K+V spread over blocks 1..QC-1 (or block 0 if
        # QC == 1); they are only needed from block QC on.
        later = []
        for b in range(1, nb):
            for c in range(cpb):
                later.append(b * cpb + c)
        tgt = list(range(1, QC)) or [0]
        for j, n in enumerate(later):
            for cl in proj_closures(n, (1, 2)):
                add(tgt[j * len(tgt) // len(later)], cl)
        # Q projections: chunk (b, c) -> block (b*QC + c) - 1, pinned to
        # mid-block windows so the drain never slips to the last window and
        # stalls the next block's first scores
        for b in range(nb):
            for c in range(cpb):
                if b == 0 and c == 0:
                    continue
                qcls = proj_closures(b * cpb + c, (0,))
                for k, cl in enumerate(qcls):
                    w = 3 + (k * min(8, KT - 4)) // max(1, len(qcls))
                    add(b * QC + c - 1, cl, dl=min(w, KT - 2))
        # AV groups of block i ride in block i+1 (even spread), each qt's
        # transpose right behind its second head; oproj token-tiles follow
        # as soon as both their groups drained. For the last block all of
        # this lands in the tail, ordered groups -> transposes -> oproj so
        # the in-order PE queue never head-of-line blocks on a DVE drain.
        for i in range(nblocks):
            b, qc = i // QC, i % QC
            in_tail = (i + 1 > nblocks - 1)
            for g in range(NG):
                add(i + 1, lambda b=b, qc=qc, g=g, d=not in_tail:
                    av_group(b, qc, g, drain=d),
                    dl=(g * KT // NG if not in_tail else None))
                if not in_tail and g % hc == hc - 1:
                    add(i + 1, lambda b=b, qc=qc, qt=g // hc:
                        av_tp(b, qc, qt), dl=g * KT // NG)
            if in_tail:
                add(i + 1, lambda b=b, qc=qc: tail_drains(b, qc))
                for qt in range(QT):
                    add(i + 1, lambda b=b, qc=qc, qt=qt: av_tp(b, qc, qt))
            for qt in range(QT):
                wr = (2 * qt + 2) * KT // NG
                tt = qc * QT + qt
                tgt_i = i + 1 if wr < KT else i + 2
                ua = tgt_i > nblocks - 1
                for j in range(ndj):
                    add(tgt_i,
                        lambda b=b, tt=tt, j=j, ua=ua: oproj_j(b, tt, j,
                                                               use_act=ua),
                        dl=(min(wr + j, KT - 1) if (wr < KT and not ua)
                            else None))

        # ---------- emission ----------
        for cl in preamble:
            cl()
        for i in range(nblocks):
            b, qc = i // QC, i % QC
            sched = [[] for _ in range(KT)]
            free = [cl for dl, cl in fillers[i] if dl is None]
            for dl, cl in fillers[i]:
                if dl is not None:
                    sched[min(dl, KT - 1)].append(cl)
            for j, cl in enumerate(free):
                sched[j * KT // max(1, len(free))].append(cl)
            for kt in range(KT):
                window(b, qc, kt)
                for cl in sched[kt]:
                    cl()
        for dl, cl in fillers[nblocks]:
            cl()

        if dbg:
            nc.sync.dma_start(out=dq2t_d,
                              in_=(q2t[:] if rowtile else q2tz[0][:]))
            nc.sync.dma_start(out=dk2t_d, in_=k2t[:])
            nc.sync.dma_start(
                out=dvnat_d.rearrange("p (b k h c) -> p b k h c",
                                      b=nb, k=KT, h=hc),
                in_=vnat[:, :, :, :, :])
            nc.sync.dma_start(out=dout2t_d, in_=out2t[:])

    nc.compile()
    return nc


_PROGRAM_CACHE = {}


def _get_program(key):
    if key not in _PROGRAM_CACHE:
        with_qkv_bias, with_o_bias = key
        _PROGRAM_CACHE[key] = build_program(
            with_qkv_bias=with_qkv_bias, with_o_bias=with_o_bias)
    return _PROGRAM_CACHE[key]


def _bf16(a):
    import ml_dtypes
    return np.asarray(a, np.float32).astype(ml_dtypes.bfloat16)


def make_in_maps(x, wq, bq, wk, bk, wv, bv, wo, bo, with_qkv_bias,
                 with_o_bias, n_cores=N_CORES, hc=HEADS_PER_CORE, dh=DEPTH):
    d2 = hc * dh
    xt = np.ascontiguousarray(_bf16(x).T)
    in_maps = []
    for c in range(n_cores):
        cs = slice(c * d2, (c + 1) * d2)
        m = {"xt": xt,
             "wq": np.ascontiguousarray(_bf16(wq)[:, cs]),
             "wk": np.ascontiguousarray(_bf16(wk)[:, cs]),
             "wv": np.ascontiguousarray(_bf16(wv)[:, cs]),
             "wo": np.ascontiguousarray(_bf16(wo)[cs, :])}
        if with_qkv_bias:
            m["bq"] = np.ascontiguousarray(
                np.asarray(bq, np.float32)[cs].reshape(d2, 1))
            m["bk"] = np.ascontiguousarray(
                np.asarray(bk, np.float32)[cs].reshape(d2, 1))
            m["bv"] = np.ascontiguousarray(
                np.asarray(bv, np.float32)[cs].reshape(d2, 1))
        if with_o_bias:
            m["bo"] = (np.asarray(bo, np.float32).reshape(1, -1) if c == 0
                       else np.zeros((1, bo.shape[-1]), np.float32))
        in_maps.append(m)
    return in_maps


def kernel(inputs, wq, bq, wk, bk, wv, bv, wo, bo):
    x = np.ascontiguousarray(np.asarray(inputs, np.float32)
                             .reshape(B_FULL * S_FULL, D_MODEL))
    with_qkv_bias = bool(np.any(bq) or np.any(bk) or np.any(bv))
    with_o_bias = bool(np.any(bo))
    nc = _get_program((with_qkv_bias, with_o_bias))

    in_maps = make_in_maps(x, wq, bq, wk, bk, wv, bv, wo, bo,
                           with_qkv_bias, with_o_bias)
    res = run_bass_kernel_spmd(nc, in_maps, list(range(N_CORES))).results
    y = np.zeros((B_FULL * S_FULL, D_MODEL), np.float64)
    for c in range(N_CORES):
        y += np.asarray(res[c]["y"], np.float32).astype(np.float64)
    return y.astype(np.float32).reshape(B_FULL, S_FULL, D_MODEL)


# revision 51
# speedup vs baseline: 1.0407x; 1.0407x over previous
"""Multi-head self-attention (B=2, S=2048, D=1024, H=16) on 8 TRN2 NeuronCores.

Sharding: head-parallel - 2 heads per core. Each core computes Q/K/V
projections for its 2 heads over all B*S tokens, full (non-causal)
softmax attention for its 4 (batch, head) units, and a partial output
projection y_c = sum_h out_h @ wo[h]. Host sums the 8 partial outputs.
The host pre-transposes x to xT and casts everything to bf16.

Device dataflow (all matmul operands bf16, PSUM accumulation fp32):
  q2t/k2t [128=2*64, T]  = w[:,2heads]^T @ xT       (PSUM accum over D)
  v --PE transpose--> vnat[k, h, 0:64] (+ ones column at col 64)
  scoresT[k, q]: per (kt, qc) window TWO concurrent row-tiled matmuls
    (head h uses array rows h*64..h*64+63 via partition-sliced APs ->
    tile_position (h*64, 0)); both heads' [128,512] tiles land in one
    2-bank PSUM tile.
  ex = exp(scale * sc) on ACT: ONE [128, 1024] activation per window.
  AV ex-stationary: po[128 q, 65] += ex_tile[128k,128q].T @ vnat[128k,65]
    accumulated over kt. Column 64 (ones) gives the softmax denominator
    per-query-ON-PARTITIONS -> reciprocal is a [128,1] DVE op (cheap).
  att[q, h*64+d] = po[:,0:64] * rci  (per-partition scalar mul)
  out2t[:, q] = PE transpose of att [128,128] (both heads at once)
  y[t, n] = out2t[:, ttile]^T @ wo   (contract 128 = 2 heads * 64)

Emission is window-scheduled: per (batch, qchunk) block there are KT
score+exp windows paced by the ACT engine (~1 us each); projection
matmuls, the previous block's AV groups, drains, transposes and output
projection tiles are spread across windows as fillers so the PE always
has work while ACT grinds through the exps.
"""

import numpy as np
from contextlib import ExitStack

import concourse.bass as bass
import concourse.tile as tile
from concourse import bacc, mybir
from concourse.bass_utils import run_bass_kernel_spmd
from concourse.masks import make_identity

F32 = mybir.dt.float32
BF = mybir.dt.bfloat16
AF = mybir.ActivationFunctionType

N_CORES = 8
D_MODEL = 1024
NUM_HEADS = 16
DEPTH = 64
HEADS_PER_CORE = NUM_HEADS // N_CORES  # 2
B_FULL = 2
S_FULL = 2048


def build_program(T=4096, D=1024, S=2048, dh=64, hc=2, with_qkv_bias=False,
                  with_o_bias=False, dbg=False, rowtile=True, ex_fp8=False):
    """Build the SPMD Bass program for one core (hc heads).

    T: total tokens (B*S); D: model dim; S: seq len per batch; dh: head
    depth; hc: heads per core. Requires hc*dh == 128, D % 128 == 0,
    S % 512 == 0, T % S == 0.
    """
    d2 = hc * dh
    assert d2 == 128 and D % 128 == 0 and S % 512 == 0 and T % S == 0
    nb = T // S            # batches
    ndc = D // 128         # D chunks (contraction tiles)
    cpb = S // 512         # 512-token chunks per batch
    KT = S // 128          # k tiles (and windows) per block
    QC = S // 512          # 512-wide q chunks (blocks) per batch
    QT = 512 // 128        # 128-wide q tiles per block
    NG = hc * QT           # AV accumulation groups per block
    NJ = min(512, D)
    ndj = D // NJ
    nch = nb * cpb         # total xt chunks
    nblocks = nb * QC
    scale = 1.0 / float(np.sqrt(dh))
    # ex dtype: fp8e4 (with FWL 4x weight load for the ex-stationary AV
    # matmuls). exp is shifted by EXB so the fp8e4 max normal (240) is never
    # hit: softmax is shift-invariant (numerator and the ones-column
    # denominator are scaled identically).
    EXDT = mybir.dt.float8e4 if ex_fp8 else BF
    EXB = -1.5 if ex_fp8 else 0.0

    nc = bacc.Bacc("TRN2", target_bir_lowering=False, debug=False,
                   num_devices=N_CORES)

    xt_d = nc.dram_tensor("xt", [D, T], BF, kind="ExternalInput").ap()
    wq_d = nc.dram_tensor("wq", [D, d2], BF, kind="ExternalInput").ap()
    wk_d = nc.dram_tensor("wk", [D, d2], BF, kind="ExternalInput").ap()
    wv_d = nc.dram_tensor("wv", [D, d2], BF, kind="ExternalInput").ap()
    wo_d = nc.dram_tensor("wo", [d2, D], BF, kind="ExternalInput").ap()
    if with_qkv_bias:
        bq_d = nc.dram_tensor("bq", [d2, 1], F32, kind="ExternalInput").ap()
        bk_d = nc.dram_tensor("bk", [d2, 1], F32, kind="ExternalInput").ap()
        bv_d = nc.dram_tensor("bv", [d2, 1], F32, kind="ExternalInput").ap()
    if with_o_bias:
        bo_d = nc.dram_tensor("bo", [1, D], F32, kind="ExternalInput").ap()
    y_d = nc.dram_tensor("y", [T, D], BF, kind="ExternalOutput").ap()
    if dbg:
        dq2t_d = nc.dram_tensor("dq2t", [128, T], BF,
                                kind="ExternalOutput").ap()
        dk2t_d = nc.dram_tensor("dk2t", [128, T], BF,
                                kind="ExternalOutput").ap()
        dvnat_d = nc.dram_tensor("dvnat", [128, nb * KT * hc * (dh + 1)], BF,
                                 kind="ExternalOutput").ap()
        dex_d = nc.dram_tensor("dex", [128, hc * 512], EXDT,
                               kind="ExternalOutput").ap()
        datt_d = nc.dram_tensor("datt", [128, 128], BF,
                                kind="ExternalOutput").ap()
        dout2t_d = nc.dram_tensor("dout2t", [128, T], BF,
                                  kind="ExternalOutput").ap()

    xt_view = xt_d.rearrange("(dc p) t -> p dc t", p=128)

    with tile.TileContext(nc) as tc, ExitStack() as ctx:
        singles = ctx.enter_context(tc.tile_pool(name="singles", bufs=1))
        xtpool = ctx.enter_context(tc.tile_pool(name="xtpool", bufs=1))
        v2pool = ctx.enter_context(tc.tile_pool(name="v2pool", bufs=2))
        expool = ctx.enter_context(tc.tile_pool(name="expool", bufs=2 * KT))
        attpool = ctx.enter_context(tc.tile_pool(name="attpool", bufs=4))
        rcpool = ctx.enter_context(tc.tile_pool(name="rcpool", bufs=4))
        ysb = ctx.enter_context(tc.tile_pool(name="ysb", bufs=6))
        # PSUM budget (8 banks): sc 2x[128,2,512]=4, po 1x[128,NG,128]=2,
        # ps 2x[128,512]=2
        scpool = ctx.enter_context(tc.tile_pool(name="sc", bufs=2,
                                                space="PSUM"))
        popool = ctx.enter_context(tc.tile_pool(name="po", bufs=1,
                                                space="PSUM"))
        pspool = ctx.enter_context(tc.tile_pool(name="ps", bufs=2,
                                                space="PSUM"))

        identb = singles.tile([128, 128], BF)
        make_identity(nc, identb[:])
        exbias = None
        if EXB != 0.0:
            exbias = singles.tile([128, 1], F32)
            nc.vector.memset(exbias[:], EXB)

        # Weights land as bf16. One dma_start = one DMA queue (~24 GB/s), so
        # ramp-critical tensors (wq/wk, first chunks) are split per-dc to
        # spread across queues, and issues alternate between the sync and
        # gpsimd sequencers (each DMA issue costs ~600ns of sequencer time).
        _eng = [nc.sync, nc.gpsimd, nc.scalar]
        _ei = [0]
        _nel = [3]

        def dma(out, in_):
            # scalar (ACT) can issue DMAs too - use it only in the preamble,
            # before the exp stream starts
            _eng[_ei[0] % _nel[0]].dma_start(out=out, in_=in_)
            _ei[0] += 1

        w_sb = []
        w_views = []
        for name, wd in (("wqs", wq_d), ("wks", wk_d), ("wvs", wv_d)):
            t = singles.tile([128, ndc, d2], BF, tag=name, name=name)
            w_sb.append(t)
            w_views.append(wd.rearrange("(dc p) m -> p dc m", p=128))
        wo_sb = singles.tile([d2, D], BF)

        b_sb = [None, None, None]
        if with_qkv_bias:
            for i, bd in enumerate((bq_d, bk_d, bv_d)):
                t = singles.tile([d2, 1], F32, tag=f"b{i}", name=f"b{i}")
                nc.sync.dma_start(out=t[:], in_=bd)
                b_sb[i] = t
        bo_sb = None
        if with_o_bias:
            bo_sb = singles.tile([128, D], F32)
            nc.gpsimd.dma_start(out=bo_sb[:], in_=bo_d.partition_broadcast(128))

        if rowtile:
            q2t = singles.tile([128, T], BF, tag="q2t")
        else:
            # padded fallback: per-head Q with the other head's rows zeroed;
            # scores contract K=128 with the full two-head K tile stationary.
            q2tz = [singles.tile([128, T], BF, tag=f"q2tz{h}",
                                 name=f"q2tz{h}") for h in range(hc)]
            for h in range(hc):
                zrows = (slice(dh, 128) if h == 0 else slice(0, h * dh))
                nc.vector.memset(q2tz[h][zrows, :], 0.0)
        k2t = singles.tile([128, T], BF, tag="k2t")
        out2t = singles.tile([128, T], BF, tag="out2t")
        # vnat[:, b, kt, h, 0:64] = V rows (k on partitions); col 64 = ones
        vnat = singles.tile([128, nb, KT, hc, dh + 1], BF, tag="vnat")
        onesc = singles.tile([128, nb, KT, hc, 1], BF)
        nc.vector.memset(onesc[:], 1.0)
        nc.vector.tensor_copy(vnat[:, :, :, :, dh:dh + 1], onesc[:])

        xts = {}     # chunk idx -> xt tile
        exs = {}     # (b, qc, kt) -> ex tile
        po_t = {}    # block idx -> po psum tile
        att_t = {}   # (b, qc, qt) -> att sbuf tile

        # PE warm-up: ~3.4us of dependency-free matmuls issued while the
        # first DMAs are in flight trips the HAM clock-gate to 2.4 GHz, so
        # the DMA-gated projection work that follows runs at full speed.
        wsc = singles.tile([128, 512], BF)
        nc.vector.memset(wsc[:], 0.0)
        wps = pspool.tile([128, 512], F32, tag="ps", name="warm")
        for _ in range(14):
            nc.tensor.matmul(wps[:], identb[:], wsc[:], start=True,
                             stop=True)

        # ---------- emission helpers ----------
        def chunk_dma(n, step=1):
            """Load xt chunk n; step=1 -> per-dc transfers (8 queues in
            parallel, for ramp-critical chunks), step=2 -> dc-pairs."""
            xt_n = xtpool.tile([128, ndc, 512], BF, tag=f"xt{n}",
                               name=f"xt{n}")
            for dc in range(0, ndc, step):
                dma(xt_n[:, dc:dc + step, :],
                    xt_view[:, dc:dc + step, n * 512:(n + 1) * 512])
            xts[n] = xt_n

        def drain_proj(p, n, ps):
            """Drain projection p of chunk n from PSUM to its home."""
            ncol = slice(n * 512, (n + 1) * 512)
            if p == 0:
                if rowtile:
                    if with_qkv_bias:
                        nc.vector.tensor_scalar_add(q2t[:, ncol], ps[:],
                                                    b_sb[0][:])
                    else:
                        nc.vector.tensor_copy(q2t[:, ncol], ps[:])
                else:
                    for h in range(hc):
                        hp = slice(h * dh, (h + 1) * dh)
                        if with_qkv_bias:
                            nc.vector.tensor_scalar_add(
                                q2tz[h][hp, ncol], ps[hp, :],
                                b_sb[0][hp, :])
                        else:
                            nc.vector.tensor_copy(q2tz[h][hp, ncol],
                                                  ps[hp, :])
            elif p == 1:
                if with_qkv_bias:
                    nc.vector.tensor_scalar_add(k2t[:, ncol], ps[:],
                                                b_sb[1][:])
                else:
                    nc.vector.tensor_copy(k2t[:, ncol], ps[:])
            else:
                v2 = v2pool.tile([128, 512], BF, tag="v2", name=f"v2_{n}")
                if with_qkv_bias:
                    nc.vector.tensor_scalar_add(v2[:], ps[:], b_sb[2][:])
                else:
                    nc.vector.tensor_copy(v2[:], ps[:])
                bb, kt0 = n // cpb, (n % cpb) * 4
                pv = pspool.tile([128, 4, 128], BF, tag="ps", name=f"pv{n}")
                for sub in range(4):
                    nc.tensor.transpose(pv[:, sub, :],
                                        v2[:, sub * 128:(sub + 1) * 128],
                                        identb[:])
                for h in range(hc):
                    nc.vector.tensor_copy(
                        vnat[:, bb, kt0:kt0 + 4, h, 0:dh],
                        pv[:, :, h * dh:(h + 1) * dh])

        def proj_closures(n, projs):
            """Per-(proj, dc) matmul closures for chunk n; drains attached
            to the last dc of each projection."""
            out = []
            for p in projs:
                state = {}

                def mk(p=p, n=n, state=state):
                    def run_dc(dc):
                        if dc == 0:
                            state["ps"] = pspool.tile(
                                [128, 512], F32, tag="ps", name=f"pj{n}_{p}")
                        nc.tensor.matmul(state["ps"][:], w_sb[p][:, dc, :],
                                         xts[n][:, dc, :],
                                         start=(dc == 0),
                                         stop=(dc == ndc - 1))
                        if dc == ndc - 1:
                            drain_proj(p, n, state["ps"])
                    return run_dc
                run_dc = mk()
                for dc in range(ndc):
                    out.append(lambda dc=dc, f=run_dc: f(dc))
            return out

        def window(b, qc, kt):
            """Score pair (row-tiled, both heads) + joint exp."""
            sc = scpool.tile([128, hc, 512], F32, tag="sc", bufs=2,
                             name=f"sc{b}_{qc}_{kt}")
            kcol = slice(b * S + kt * 128, b * S + (kt + 1) * 128)
            qcol = slice(b * S + qc * 512, b * S + (qc + 1) * 512)
            for h in range(hc):
                if rowtile:
                    hp = slice(h * dh, (h + 1) * dh)
                    nc.tensor.matmul(sc[:, h, :], k2t[hp, kcol],
                                     q2t[hp, qcol], start=True, stop=True)
                else:
                    nc.tensor.matmul(sc[:, h, :], k2t[:, kcol],
                                     q2tz[h][:, qcol], start=True, stop=True)
            ex = expool.tile([128, hc, 512], EXDT, tag="ex", bufs=2 * KT,
                             name=f"ex{b}_{qc}_{kt}")
            nc.scalar.activation(ex[:, :, :], sc[:, :, :], AF.Exp,
                                 scale=scale,
                                 bias=(exbias[:] if exbias is not None
                                       else 0.0))
            exs[(b, qc, kt)] = ex
            if dbg and (b, qc, kt) == (0, 0, 0):
                nc.sync.dma_start(
                    out=dex_d.rearrange("p (h q) -> p h q", h=hc),
                    in_=ex[:, :, :])

        def av_group(b, qc, g, drain=True):
            """AV accumulation group g = qt*hc + h of block (b, qc), plus
            its drain. Consecutive groups alternate between two 1-bank po
            tiles so a group's matmuls never pick up a false (tile-granular)
            dependency on the previous group's DVE drain."""
            i = b * QC + qc
            if i not in po_t:
                po_t[i] = [popool.tile([128, NG // 2, 128], F32,
                                       tag=f"po{p}", name=f"po{p}_{i}")
                           for p in range(2)]
            po = po_t[i][g % 2]
            gs = g // 2
            qt, h = g // hc, g % hc
            for kt in range(KT):
                nc.tensor.matmul(
                    po[:, gs, 0:dh + 1],
                    exs[(b, qc, kt)][:, h, qt * 128:(qt + 1) * 128],
                    vnat[:, b, kt, h, :],
                    start=(kt == 0), stop=(kt == KT - 1))
            if drain:
                rci = rcpool.tile([128, 1], F32, tag="rci", bufs=4,
                                  name=f"rci{i}_{g}")
                nc.vector.reciprocal(rci[:], po[:, gs, dh:dh + 1])
                att = get_att(b, qc, qt)
                nc.vector.tensor_scalar_mul(att[:, h * dh:(h + 1) * dh],
                                            po[:, gs, 0:dh], rci[:])

        def get_att(b, qc, qt):
            i = b * QC + qc
            if (b, qc, qt) not in att_t:
                att_t[(b, qc, qt)] = attpool.tile(
                    [128, 128], BF, tag="att", bufs=4, name=f"att{i}_{qt}")
            return att_t[(b, qc, qt)]

        def tail_drains(b, qc):
            """Batched drain of all NG groups: one strided reciprocal per po
            tile, then per-group normalize-muls on the (now idle) ACT engine
            interleaved with DVE - only used in the tail."""
            i = b * QC + qc
            rci = rcpool.tile([128, 2, NG // 2, 1], F32, tag="rcib", bufs=1,
                              name=f"rcib{i}")
            for p in range(2):
                nc.vector.reciprocal(rci[:, p, :, :],
                                     po_t[i][p][:, :, dh:dh + 1])
            for g in range(NG):
                po = po_t[i][g % 2]
                gs = g // 2
                qt, h = g // hc, g % hc
                att = get_att(b, qc, qt)
                if g % 2 == 0:
                    nc.scalar.activation(att[:, h * dh:(h + 1) * dh],
                                         po[:, gs, 0:dh], AF.Copy,
                                         scale=rci[:, g % 2, gs, :])
                else:
                    nc.vector.tensor_scalar_mul(att[:, h * dh:(h + 1) * dh],
                                                po[:, gs, 0:dh],
                                                rci[:, g % 2, gs, :])

        def av_tp(b, qc, qt):
            """Transpose the finished two-head att tile into out2t."""
            i = b * QC + qc
            att = att_t[(b, qc, qt)]
            if dbg and (b, qc, qt) == (0, 0, 0):
                nc.sync.dma_start(out=datt_d, in_=att[:])
            q0 = qc * 512 + qt * 128
            pt = pspool.tile([128, 128], BF, tag="ps", name=f"pt{i}_{qt}")
            nc.tensor.transpose(pt[:], att[:], identb[:])
            nc.vector.tensor_copy(out2t[:, b * S + q0:b * S + q0 + 128],
                                  pt[:])

        _tpy = [0]

        def oproj_j(b, tt, j, use_act=False):
            """One n-chunk of the output projection for token tile tt."""
            tcol = slice(b * S + tt * 128, b * S + (tt + 1) * 128)
            trow = slice(b * S + tt * 128, b * S + (tt + 1) * 128)
            if True:
                if use_act:
                    # tail: the AV po banks are free - rotate py over
                    # ps/po0/po1 so the oproj matmuls don't drain-serialize
                    tags = ("ps", "po0", "po1")
                    tag = tags[_tpy[0] % 3]
                    _tpy[0] += 1
                    pool = pspool if tag == "ps" else popool
                else:
                    tag, pool = "ps", pspool
                py = pool.tile([128, NJ], F32, tag=tag,
                               name=f"py{b}_{tt}_{j}")
                nc.tensor.matmul(py[:], out2t[:, tcol],
                                 wo_sb[:, j * NJ:(j + 1) * NJ],
                                 start=True, stop=True)
                yt = ysb.tile([128, NJ], BF, tag="yt", name=f"yt{b}_{tt}_{j}")
                if with_o_bias:
                    nc.vector.tensor_add(yt[:], py[:],
                                         bo_sb[:, j * NJ:(j + 1) * NJ])
                elif use_act and j % 2 == 0:
                    # tail only: ACT is idle once the exps are done; split
                    # the PSUM->SBUF drains between ACT and DVE so the py
                    # psum ring never serializes on a single engine
                    nc.scalar.copy(yt[:], py[:])
                else:
                    nc.vector.tensor_copy(yt[:], py[:])
                # two transfers on two DMA queues halve the tile's latency
                hj = NJ // 2
                nc.gpsimd.dma_start(
                    out=y_d[trow, j * NJ:j * NJ + hj], in_=yt[:, 0:hj])
                nc.sync.dma_start(
                    out=y_d[trow, j * NJ + hj:(j + 1) * NJ], in_=yt[:, hj:NJ])

        # ---------- static schedule ----------
        # fillers[i] = list of (deadline_window_or_None, closure) for block i;
        # fillers[nblocks] is the tail.
        fillers = [[] for _ in range(nblocks + 1)]

        def add(i, cl, dl=None):
            fillers[min(i, nblocks)].append((dl, cl))

        # All input DMAs issue up front, in need-order: chunk 0/1 and wv
        # per-dc (ramp critical), later chunks coarser. Chunk 0 K+Q compute
        # is in the preamble (needed by window (0,0,0)); chunk-0 V rides
        # inside block 0 (first needed by the AV groups during block 1).
        preamble = []

        def ramp_dma():
            # wq/wk (per dc-pair) interleaved with chunk-0 (per half-dc,
            # 64KB pieces ~2.9us each on a queue) so the first projection
            # matmuls unblock as early as possible
            xt0 = xtpool.tile([128, ndc, 512], BF, tag="xt0", name="xt0")
            xts[0] = xt0
            for dc in range(0, ndc, 2):
                for w in (0, 1):
                    dma(w_sb[w][:, dc:dc + 2, :], w_views[w][:, dc:dc + 2, :])
                for d in (dc, dc + 1):
                    dma(xt0[:, d, 0:256], xt_view[:, d, 0:256])
                    dma(xt0[:, d, 256:512], xt_view[:, d, 256:512])
        preamble.append(ramp_dma)
        if nch > 1:
            preamble.append(lambda: chunk_dma(1, step=1))

        def wv_wo_dma():
            _nel[0] = 2    # stop using the scalar queue; exps start soon
            for dc in range(0, ndc, 2):
                dma(w_sb[2][:, dc:dc + 2, :], w_views[2][:, dc:dc + 2, :])
            dma(wo_sb[:], wo_d)
        preamble.append(wv_wo_dma)
        for n in range(2, nch):
            preamble.append(lambda n=n: chunk_dma(n, step=2))
        # interleave chunk-0 K and Q matmuls per-dc: both consume the same
        # arriving xt piece, so each DMA arrival feeds two matmuls instead
        # of the Q chain re-waiting for pieces K already used (the two
        # accumulators sit in different PSUM banks, so the interleaved
        # groups stay independent)
        for ka, qa in zip(proj_closures(0, (1,)), proj_closures(0, (0,))):
            preamble.append(ka)
            preamble.append(qa)
        for cl in proj_closures(0, (2,)):
            add(0, cl)

        # batch-0 chunks 1..cpb-1: K+V into block 0 with per-window
        # deadlines (chunk c's keys are first needed at window c*KT//cpb);
        # Q-projection of chunk (b, c) goes to the block before (b, c).
        kpc = max(1, KT // cpb)
        for c in range(1, cpb):
            kv = proj_closures(c, (1, 2))
            ws = list(range((c - 1) * kpc, c * kpc))
            for j, cl in enumerate(kv):
                add(0, cl, dl=ws[j * len(ws) // len(kv)])
        # batch>=1 chunks: K+V spread over blocks 1..QC-1 (or block 0 if
        # QC == 1); they are only needed from block QC on.
        later = []
        for b in range(1, nb):
            for c in range(cpb):
                later.append(b * cpb + c)
        tgt = list(range(1, QC)) or [0]
        for j, n in enumerate(later):
            for cl in proj_closures(n, (1, 2)):
                add(tgt[j * len(tgt) // len(later)], cl)
        # Q projections: chunk (b, c) -> block (b*QC + c) - 1
        for b in range(nb):
            for c in range(cpb):
                if b == 0 and c == 0:
                    continue
                for cl in proj_closures(b * cpb + c, (0,)):
                    add(b * QC + c - 1, cl)
        # AV groups of block i ride in block i+1 (even spread), each qt's
        # transpose right behind its second head; oproj token-tiles follow
        # as soon as both their groups drained. For the last block all of
        # this lands in the tail, ordered groups -> transposes -> oproj so
        # the in-order PE queue never head-of-line blocks on a DVE drain.
        for i in range(nblocks):
            b, qc = i // QC, i % QC
            in_tail = (i + 1 > nblocks - 1)
            for g in range(NG):
                add(i + 1, lambda b=b, qc=qc, g=g, d=not in_tail:
                    av_group(b, qc, g, drain=d),
                    dl=(g * KT // NG if not in_tail else None))
                if not in_tail and g % hc == hc - 1:
                    add(i + 1, lambda b=b, qc=qc, qt=g // hc:
                        av_tp(b, qc, qt), dl=g * KT // NG)
            if in_tail:
                add(i + 1, lambda b=b, qc=qc: tail_drains(b, qc))
                for qt in range(QT):
                    add(i + 1, lambda b=b, qc=qc, qt=qt: av_tp(b, qc, qt))
            for qt in range(QT):
                wr = (2 * qt + 2) * KT // NG
                tt = qc * QT + qt
                tgt_i = i + 1 if wr < KT else i + 2
                ua = tgt_i > nblocks - 1
                for j in range(ndj):
                    add(tgt_i,
                        lambda b=b, tt=tt, j=j, ua=ua: oproj_j(b, tt, j,
                                                               use_act=ua),
                        dl=(min(wr + j, KT - 1) if (wr < KT and not ua)
                            else None))

        # ---------- emission ----------
        for cl in preamble:
            cl()
        for i in range(nblocks):
            b, qc = i // QC, i % QC
            sched = [[] for _ in range(KT)]
            free = [cl for dl, cl in fillers[i] if dl is None]
            for dl, cl in fillers[i]:
                if dl is not None:
                    sched[min(dl, KT - 1)].append(cl)
            for j, cl in enumerate(free):
                sched[j * KT // max(1, len(free))].append(cl)
            for kt in range(KT):
                window(b, qc, kt)
                for cl in sched[kt]:
                    cl()
        for dl, cl in fillers[nblocks]:
            cl()

        if dbg:
            nc.sync.dma_start(out=dq2t_d,
                              in_=(q2t[:] if rowtile else q2tz[0][:]))
            nc.sync.dma_start(out=dk2t_d, in_=k2t[:])
            nc.sync.dma_start(
                out=dvnat_d.rearrange("p (b k h c) -> p b k h c",
                                      b=nb, k=KT, h=hc),
                in_=vnat[:, :, :, :, :])
            nc.sync.dma_start(out=dout2t_d, in_=out2t[:])

    nc.compile()
    return nc


_PROGRAM_CACHE = {}


def _get_program(key):
    if key not in _PROGRAM_CACHE:
        with_qkv_bias, with_o_bias = key
        _PROGRAM_CACHE[key] = build_program(
            with_qkv_bias=with_qkv_bias, with_o_bias=with_o_bias)
    return _PROGRAM_CACHE[key]


def _bf16(a):
    import ml_dtypes
    return np.asarray(a, np.float32).astype(ml_dtypes.bfloat16)


def make_in_maps(x, wq, bq, wk, bk, wv, bv, wo, bo, with_qkv_bias,
                 with_o_bias, n_cores=N_CORES, hc=HEADS_PER_CORE, dh=DEPTH):
    d2 = hc * dh
    xt = np.ascontiguousarray(_bf16(x).T)
    in_maps = []
    for c in range(n_cores):
        cs = slice(c * d2, (c + 1) * d2)
        m = {"xt": xt,
             "wq": np.ascontiguousarray(_bf16(wq)[:, cs]),
             "wk": np.ascontiguousarray(_bf16(wk)[:, cs]),
             "wv": np.ascontiguousarray(_bf16(wv)[:, cs]),
             "wo": np.ascontiguousarray(_bf16(wo)[cs, :])}
        if with_qkv_bias:
            m["bq"] = np.ascontiguousarray(
                np.asarray(bq, np.float32)[cs].reshape(d2, 1))
            m["bk"] = np.ascontiguousarray(
                np.asarray(bk, np.float32)[cs].reshape(d2, 1))
            m["bv"] = np.ascontiguousarray(
                np.asarray(bv, np.float32)[cs].reshape(d2, 1))
        if with_o_bias:
            m["bo"] = (np.asarray(bo, np.float32).reshape(1, -1) if c == 0
                       else np.zeros((1, bo.shape[-1]), np.float32))
        in_maps.append(m)
    return in_maps


def kernel(inputs, wq, bq, wk, bk, wv, bv, wo, bo):
    x = np.ascontiguousarray(np.asarray(inputs, np.float32)
                             .reshape(B_FULL * S_FULL, D_MODEL))
    with_qkv_bias = bool(np.any(bq) or np.any(bk) or np.any(bv))
    with_o_bias = bool(np.any(bo))
    nc = _get_program((with_qkv_bias, with_o_bias))

    in_maps = make_in_maps(x, wq, bq, wk, bk, wv, bv, wo, bo,
                           with_qkv_bias, with_o_bias)
    res = run_bass_kernel_spmd(nc, in_maps, list(range(N_CORES))).results
    y = np.zeros((B_FULL * S_FULL, D_MODEL), np.float64)
    for c in range(N_CORES):
        y += np.asarray(res[c]["y"], np.float32).astype(np.float64)
    return y.astype(np.float32).reshape(B_FULL, S_FULL, D_MODEL)
